# revision 2
# baseline (speedup 1.0000x reference)
"""Multi-level ROI Align (FPN pooler, 4 levels summed) on 8 Trainium2 cores.

Strategy: shard ROIs across cores (core k: batch k//4, 128 ROIs). All gather
indices and bilinear weights are computed on host from `boxes`; the device
kernel does the heavy lifting: HBM pixel gathers (dma_gather) + weighted
scatter-reduction into 7x7 bins via PSUM-accumulating matmuls.

Per ROI, per level:
  out[bin, c] = sum_j W[j, bin] * G[j, c]
where G rows are gathered pixel vectors (C=256) and W is sparse (built on
device as fixed_pattern * per-partition scalar for L0/L1, host-baked dense
for the region-gathered L2/L3).

L0 uses 3-pixel elements addressed at even-pixel granularity (idx = flat//2)
to fit the int16 index range (200*200 = 40000 > 32767).
"""
import sys
import numpy as np

sys.path.insert(0, '/opt/trn_rl_repo')

POOLED = 7
SAMP = 2
NBIN = 49
C = 256
IMG = 800.0

# per level: H, W, scale, mode
#   mode 'tri': 3-px elems, idx=flat//2, NJ j's with 3 weight slots
#   mode 'px' : 1-px elems, corner gathers
#   mode 'reg': 1-px elems, bounding-region pixels, host-baked lhsT
L0 = dict(H=200, W=200, scale=0.25, mode='tri', NJ=512, REAL=392, NCH=4)
L1 = dict(H=100, W=100, scale=0.125, mode='px', NJ=896, REAL=784, NCH=7)
L2 = dict(H=50, W=50, scale=0.0625, mode='reg', NJ=384, REAL=324, NCH=3, WREG=18)
L3 = dict(H=25, W=25, scale=0.03125, mode='reg', NJ=128, REAL=100, NCH=1, WREG=10)
LEVELS = [L0, L1, L2, L3]

NROI_CORE = 128     # ROIs per core
NGRP = 64           # groups of 2 ROIs
GRP = 2

# padded flat pixel counts of the feature buffers
F0_ROWS = 40004     # covers 3-px elem overrun
F1_ROWS = 10000
F2_ROWS = 3400      # covers region overrun (y,x up to 66)
F3_ROWS = 900       # covers region overrun (y,x up to 33)

# const fp32 column layout (per partition)
PAT0_OFF = 0                       # [4, 49]
PAT1_OFF = PAT0_OFF + 4 * NBIN     # [7, 49]
WCOL0_OFF = PAT1_OFF + 7 * NBIN    # [128 roi * 12]
WCOL1_OFF = WCOL0_OFF + NROI_CORE * 12   # [128 roi * 7]
ID_OFF = WCOL1_OFF + NROI_CORE * 7       # [49]
CST_COLS = ID_OFF + NBIN

# idx int16 column layout (per partition), per 2-ROI group
IC0, IC1, IC2, IC3 = 64, 112, 48, 16     # cols per group per level
IDX0_OFF = 0
IDX1_OFF = IDX0_OFF + NGRP * IC0
IDX2_OFF = IDX1_OFF + NGRP * IC1
IDX3_OFF = IDX2_OFF + NGRP * IC2
IDX_COLS = IDX3_OFF + NGRP * IC3

_MODULE_CACHE = {}


def _sample_meta(boxes_b, H, W, scale):
    """Per-ROI sample geometry in fp32, matching reference op order.
    boxes_b: [N, 4] fp32. Returns dict of [N,7,2] arrays."""
    f = np.float32
    b = boxes_b.astype(np.float32)
    x1 = b[:, 0] * f(scale)
    y1 = b[:, 1] * f(scale)
    x2 = b[:, 2] * f(scale)
    y2 = b[:, 3] * f(scale)
    rw = np.maximum(x2 - x1, f(1.0))
    rh = np.maximum(y2 - y1, f(1.0))
    bw = rw / f(POOLED)
    bh = rh / f(POOLED)
    g = (np.arange(POOLED, dtype=np.float32)[:, None]
         + (np.arange(SAMP, dtype=np.float32)[None, :] + f(0.5)) / f(SAMP))
    y = y1[:, None, None] + g[None] * bh[:, None, None]   # [N,7,2]
    x = x1[:, None, None] + g[None] * bw[:, None, None]
    masky = ((y >= f(-1.0)) & (y <= f(H))).astype(np.float32)
    maskx = ((x >= f(-1.0)) & (x <= f(W))).astype(np.float32)
    yc = np.clip(y, f(0.0), f(H - 1))
    xc = np.clip(x, f(0.0), f(W - 1))
    yl = np.floor(yc).astype(np.int64)
    xl = np.floor(xc).astype(np.int64)
    yh = np.minimum(yl + 1, H - 1)
    xh = np.minimum(xl + 1, W - 1)
    ly = (yc - yl.astype(np.float32)).astype(np.float32)
    lx = (xc - xl.astype(np.float32)).astype(np.float32)
    hy = (f(1.0) - ly).astype(np.float32)
    hx = (f(1.0) - lx).astype(np.float32)
    return dict(yl=yl, yh=yh, xl=xl, xh=xh, ly=ly, lx=lx, hy=hy, hx=hx,
                masky=masky, maskx=maskx, x=x, y=y)


def _build_tri(meta, lv):
    """L0: j = (row_sel, py, sy, px, sx) -> 392 3-px elems, 3 slot weights.
    Returns idx [N, NJ] int64, w [N, NJ, 3] fp32."""
    N = meta['yl'].shape[0]
    W = lv['W']
    NJ, REAL = lv['NJ'], lv['REAL']
    rows = np.stack([meta['yl'], meta['yh']], axis=1)          # [N,2,7,2] (rs)
    wys = np.stack([meta['hy'], meta['ly']], axis=1)           # [N,2,7,2]
    m = (meta['masky'][:, :, :, None, None] * meta['maskx'][:, None, None, :, :])  # [N,7,2,7,2]
    # broadcast to [N, rs, py, sy, px, sx]
    row = np.broadcast_to(rows[:, :, :, :, None, None], (N, 2, 7, 2, 7, 2))
    wy = np.broadcast_to(wys[:, :, :, :, None, None], (N, 2, 7, 2, 7, 2)).astype(np.float32)
    xl = np.broadcast_to(meta['xl'][:, None, None, None, :, :], (N, 2, 7, 2, 7, 2))
    hx = np.broadcast_to(meta['hx'][:, None, None, None, :, :], (N, 2, 7, 2, 7, 2)).astype(np.float32)
    lx = np.broadcast_to(meta['lx'][:, None, None, None, :, :], (N, 2, 7, 2, 7, 2)).astype(np.float32)
    mm = np.broadcast_to(m[:, None], (N, 2, 7, 2, 7, 2)).astype(np.float32)
    flat = row * W + xl
    idx = (flat >> 1).reshape(N, REAL)
    r = (flat & 1).astype(np.float32).reshape(N, REAL)
    wl = (wy * hx * mm * np.float32(0.25)).reshape(N, REAL)
    wh = (wy * lx * mm * np.float32(0.25)).reshape(N, REAL)
    w = np.zeros((N, NJ, 3), np.float32)
    w[:, :REAL, 0] = wl * (1 - r)
    w[:, :REAL, 1] = wl * r + wh * (1 - r)
    w[:, :REAL, 2] = wh * r
    idx_full = np.zeros((N, NJ), np.int64)
    idx_full[:, :REAL] = idx
    return idx_full, w


def _build_px(meta, lv):
    """L1: j = (row_sel, col_sel, py, sy, px, sx) -> 784 1-px corner gathers.
    Returns idx [N, NJ] int64, w [N, NJ] fp32."""
    N = meta['yl'].shape[0]
    W = lv['W']
    NJ, REAL = lv['NJ'], lv['REAL']
    rows = np.stack([meta['yl'], meta['yh']], axis=1)   # [N,2(rs),7,2]
    wys = np.stack([meta['hy'], meta['ly']], axis=1)
    cols = np.stack([meta['xl'], meta['xh']], axis=1)   # [N,2(cs),7,2]
    wxs = np.stack([meta['hx'], meta['lx']], axis=1)
    m = (meta['masky'][:, :, :, None, None] * meta['maskx'][:, None, None, :, :])
    row = np.broadcast_to(rows[:, :, None, :, :, None, None], (N, 2, 2, 7, 2, 7, 2))
    wy = np.broadcast_to(wys[:, :, None, :, :, None, None], (N, 2, 2, 7, 2, 7, 2)).astype(np.float32)
    col = np.broadcast_to(cols[:, None, :, None, None, :, :], (N, 2, 2, 7, 2, 7, 2))
    wx = np.broadcast_to(wxs[:, None, :, None, None, :, :], (N, 2, 2, 7, 2, 7, 2)).astype(np.float32)
    mm = np.broadcast_to(m[:, None, None], (N, 2, 2, 7, 2, 7, 2)).astype(np.float32)
    idx = (row * W + col).reshape(N, REAL)
    w = (wy * wx * mm * np.float32(0.25)).reshape(N, REAL)
    idx_full = np.zeros((N, NJ), np.int64)
    w_full = np.zeros((N, NJ), np.float32)
    idx_full[:, :REAL] = idx
    w_full[:, :REAL] = w
    return idx_full, w_full


def _build_reg(meta, lv):
    """L2/L3: bounding-region pixels + separable host-baked weights.
    Returns idx [N, NJ] int64, lhsT [N, NJ, 49] fp32."""
    N = meta['yl'].shape[0]
    H, W, WREG = lv['H'], lv['W'], lv['WREG']
    NJ, REAL = lv['NJ'], lv['REAL']
    f = np.float32
    y_base = np.floor(np.clip(meta['y'].reshape(N, -1).min(1), 0.0, H - 1)).astype(np.int64)
    x_base = np.floor(np.clip(meta['x'].reshape(N, -1).min(1), 0.0, W - 1)).astype(np.int64)
    # WY [N, WREG, 7], WX [N, WREG, 7]
    WY = np.zeros((N, WREG, POOLED), np.float32)
    WX = np.zeros((N, WREG, POOLED), np.float32)
    ridx = np.arange(N)[:, None, None]
    pidx = np.broadcast_to(np.arange(POOLED)[None, :, None], (N, POOLED, SAMP))
    np.add.at(WY, (ridx, meta['yl'] - y_base[:, None, None], pidx),
              (f(0.5) * meta['hy'] * meta['masky']).astype(np.float32))
    np.add.at(WY, (ridx, meta['yh'] - y_base[:, None, None], pidx),
              (f(0.5) * meta['ly'] * meta['masky']).astype(np.float32))
    np.add.at(WX, (ridx, meta['xl'] - x_base[:, None, None], pidx),
              (f(0.5) * meta['hx'] * meta['maskx']).astype(np.float32))
    np.add.at(WX, (ridx, meta['xh'] - x_base[:, None, None], pidx),
              (f(0.5) * meta['lx'] * meta['maskx']).astype(np.float32))
    lhsT = np.einsum('nap,nbq->nabpq', WY, WX).reshape(N, REAL, NBIN)
    dy = np.arange(WREG)
    idx = ((y_base[:, None, None] + dy[None, :, None]) * W
           + x_base[:, None, None] + dy[None, None, :]).reshape(N, REAL)
    idx_full = np.zeros((N, NJ), np.int64)
    lhsT_full = np.zeros((N, NJ, NBIN), np.float32)
    idx_full[:, :REAL] = idx
    lhsT_full[:, :REAL] = lhsT
    return idx_full, lhsT_full


def _pack_idx(jlists):
    """Pack concatenated per-group idx list [NJ_total] -> [128, NJ_total//16]
    int16 wrapped in 16 partitions, replicated 8x."""
    jl = np.asarray(jlists)
    n = jl.shape[-1]
    arr = jl.reshape(*jl.shape[:-1], n // 16, 16)   # [..., col, p]
    arr = np.swapaxes(arr, -1, -2)                  # [..., p(16), col]
    arr = np.broadcast_to(arr[..., None, :, :],
                          (*jl.shape[:-1], 8, 16, n // 16))
    return arr.reshape(*jl.shape[:-1], 128, n // 16).astype(np.int16)


def _bin_pattern(mode, NCH, REAL):
    """Fixed j->bin one-hot pattern [128, NCH, 49] for 'tri'/'px' j order."""
    NJ = NCH * 128
    j = np.arange(NJ)
    if mode == 'tri':
        # j = ((((rs*7+py)*2+sy)*7+px)*2+sx)
        px = (j // 2) % 7
        py = (j // (2 * 7 * 2)) % 7
    else:
        # j = (((((rs*2+cs)*7+py)*2+sy)*7+px)*2+sx)
        px = (j // 2) % 7
        py = (j // (2 * 7 * 2)) % 7
    bins = py * 7 + px
    pat = np.zeros((NJ, NBIN), np.float32)
    valid = j < REAL
    pat[np.arange(NJ)[valid], bins[valid]] = 1.0
    return pat.reshape(NCH, 128, NBIN).transpose(1, 0, 2)   # [128, NCH, 49]


def _host_prepare(x0, x1, x2, x3, boxes):
    """Build all per-core input tensors. Returns list of 8 dicts."""
    B = boxes.shape[0]
    feats = []
    for arr, lv, rows in ((x0, L0, F0_ROWS), (x1, L1, F1_ROWS),
                          (x2, L2, F2_ROWS), (x3, L3, F3_ROWS)):
        f = np.zeros((B, rows, C), np.float32)
        hw = lv['H'] * lv['W']
        f[:, :hw] = np.ascontiguousarray(
            np.transpose(np.asarray(arr, np.float32), (0, 2, 3, 1))).reshape(B, hw, C)
        feats.append(f)

    per_batch = []
    for b in range(B):
        bb = np.asarray(boxes[b], np.float32)
        m0 = _sample_meta(bb, L0['H'], L0['W'], L0['scale'])
        m1 = _sample_meta(bb, L1['H'], L1['W'], L1['scale'])
        m2 = _sample_meta(bb, L2['H'], L2['W'], L2['scale'])
        m3 = _sample_meta(bb, L3['H'], L3['W'], L3['scale'])
        idx0, w0 = _build_tri(m0, L0)
        idx1, w1 = _build_px(m1, L1)
        idx2, lt2 = _build_reg(m2, L2)
        idx3, lt3 = _build_reg(m3, L3)
        per_batch.append((idx0, w0, idx1, w1, idx2, lt2, idx3, lt3))

    pat0 = _bin_pattern('tri', L0['NCH'], L0['REAL'])
    pat1 = _bin_pattern('px', L1['NCH'], L1['REAL'])

    in_maps = []
    for k in range(8):
        b = k // 4
        s = (k % 4) * NROI_CORE
        idx0, w0, idx1, w1, idx2, lt2, idx3, lt3 = per_batch[b]
        sl = slice(s, s + NROI_CORE)

        cst = np.zeros((128, CST_COLS), np.float32)
        cst[:, PAT0_OFF:PAT0_OFF + 4 * NBIN] = pat0.reshape(128, -1)
        cst[:, PAT1_OFF:PAT1_OFF + 7 * NBIN] = pat1.reshape(128, -1)
        # wcol0 [128, roi*12]: col roi*12 + c*3 + slot = w0[roi, c*128+p, slot]
        wc0 = w0[sl].reshape(NROI_CORE, L0['NCH'], 128, 3)   # [roi,c,p,s]
        cst[:, WCOL0_OFF:WCOL0_OFF + NROI_CORE * 12] = (
            wc0.transpose(2, 0, 1, 3).reshape(128, -1))
        wc1 = w1[sl].reshape(NROI_CORE, L1['NCH'], 128)      # [roi,c,p]
        cst[:, WCOL1_OFF:WCOL1_OFF + NROI_CORE * 7] = (
            wc1.transpose(2, 0, 1).reshape(128, -1))
        cst[:NBIN, ID_OFF:ID_OFF + NBIN] = np.eye(NBIN, dtype=np.float32)

        idxs = np.zeros((128, IDX_COLS), np.int16)
        idxs[:, IDX0_OFF:IDX0_OFF + NGRP * IC0] = _pack_idx(
            idx0[sl].reshape(NGRP, GRP * L0['NJ'])).transpose(1, 0, 2).reshape(128, -1)
        idxs[:, IDX1_OFF:IDX1_OFF + NGRP * IC1] = _pack_idx(
            idx1[sl].reshape(NGRP, GRP * L1['NJ'])).transpose(1, 0, 2).reshape(128, -1)
        idxs[:, IDX2_OFF:IDX2_OFF + NGRP * IC2] = _pack_idx(
            idx2[sl].reshape(NGRP, GRP * L2['NJ'])).transpose(1, 0, 2).reshape(128, -1)
        idxs[:, IDX3_OFF:IDX3_OFF + NGRP * IC3] = _pack_idx(
            idx3[sl].reshape(NGRP, GRP * L3['NJ'])).transpose(1, 0, 2).reshape(128, -1)

        # lhsT k-major: lt2 [roi, NJ(=3*128), 49] -> [roi, 128, 3, 49]
        lt2k = np.ascontiguousarray(
            lt2[sl].reshape(NROI_CORE, L2['NCH'], 128, NBIN).transpose(0, 2, 1, 3))
        lt3k = np.ascontiguousarray(lt3[sl].reshape(NROI_CORE, 128, NBIN))

        in_maps.append({
            "f0": feats[0][b], "f1": feats[1][b],
            "f2": feats[2][b], "f3": feats[3][b],
            "cst": cst, "idxs": idxs, "lt2": lt2k, "lt3": lt3k,
        })
    return in_maps


def _build_module():
    from concourse import bacc, tile
    from concourse.bass import mybir
    import concourse.bass as bass_mod

    F32 = mybir.dt.float32
    I16 = mybir.dt.int16
    AP = bass_mod.AP

    nc = bacc.Bacc(None, target_bir_lowering=False)
    f0 = nc.dram_tensor("f0", [F0_ROWS, C], F32, kind="ExternalInput")
    f1 = nc.dram_tensor("f1", [F1_ROWS, C], F32, kind="ExternalInput")
    f2 = nc.dram_tensor("f2", [F2_ROWS, C], F32, kind="ExternalInput")
    f3 = nc.dram_tensor("f3", [F3_ROWS, C], F32, kind="ExternalInput")
    cst = nc.dram_tensor("cst", [128, CST_COLS], F32, kind="ExternalInput")
    idxs = nc.dram_tensor("idxs", [128, IDX_COLS], I16, kind="ExternalInput")
    lt2 = nc.dram_tensor("lt2", [NROI_CORE, 128, L2['NCH'], NBIN], F32, kind="ExternalInput")
    lt3 = nc.dram_tensor("lt3", [NROI_CORE, 128, NBIN], F32, kind="ExternalInput")
    out = nc.dram_tensor("out", [NROI_CORE, C, NBIN], F32, kind="ExternalOutput")

    # overlapping 3-px elem view of f0: stride 2px, width 3px
    f0_view = AP(f0, 0, [[2 * C, F0_ROWS // 2 - 1], [1, 3 * C]])
    gather_srcs = [f0_view, f1[:], f2[:], f3[:]]
    ELEM = [3 * C, C, C, C]
    STEP = [2 * C, C, C, C]
    ICOLS = [IC0, IC1, IC2, IC3]
    IOFF = [IDX0_OFF, IDX1_OFF, IDX2_OFF, IDX3_OFF]

    with tile.TileContext(nc) as tc:
        with (
            tc.tile_pool(name="const", bufs=1) as constp,
            tc.tile_pool(name="g0p", bufs=2) as g0p,
            tc.tile_pool(name="g1p", bufs=2) as g1p,
            tc.tile_pool(name="g2p", bufs=2) as g2p,
            tc.tile_pool(name="g3p", bufs=2) as g3p,
            tc.tile_pool(name="ltp", bufs=3) as ltp,
            tc.tile_pool(name="wp", bufs=6) as wp,
            tc.tile_pool(name="accp", bufs=4, space="PSUM") as accp,
            tc.tile_pool(name="ptp", bufs=2, space="PSUM") as ptp,
            tc.tile_pool(name="evp", bufs=3) as evp,
            tc.tile_pool(name="otp", bufs=3) as otp,
        ):
            cst_t = constp.tile([128, CST_COLS], F32)
            nc.sync.dma_start(cst_t[:], cst[:])
            idx_t = constp.tile([128, IDX_COLS], I16)
            nc.sync.dma_start(idx_t[:], idxs[:])

            gpools = [g0p, g1p, g2p, g3p]
            for grp in range(NGRP):
                gts = []
                for l, lv in enumerate(LEVELS):
                    nidx = GRP * lv['NJ']
                    gt = gpools[l].tile([128, GRP * lv['NCH'], ELEM[l]], F32,
                                        tag=f"g{l}")
                    io = IOFF[l] + grp * ICOLS[l]
                    if nidx <= 1024:
                        nc.gpsimd.dma_gather(
                            gt[:], gather_srcs[l], idx_t[:, io:io + ICOLS[l]],
                            nidx, nidx, ELEM[l], elem_step=STEP[l])
                    else:
                        # SWDGE ring cap: split into one call per ROI
                        hc = ICOLS[l] // GRP
                        for r2 in range(GRP):
                            nc.gpsimd.dma_gather(
                                gt[:, r2 * lv['NCH']:(r2 + 1) * lv['NCH'], :],
                                gather_srcs[l],
                                idx_t[:, io + r2 * hc:io + (r2 + 1) * hc],
                                lv['NJ'], lv['NJ'], ELEM[l], elem_step=STEP[l])
                    gts.append(gt)

                for r2 in range(GRP):
                    roi = grp * GRP + r2
                    lt2_t = ltp.tile([128, L2['NCH'], NBIN], F32, tag="lt2")
                    nc.sync.dma_start(lt2_t[:], lt2[roi])
                    lt3_t = ltp.tile([128, NBIN], F32, tag="lt3")
                    nc.sync.dma_start(lt3_t[:], lt3[roi])

                    acc = accp.tile([NBIN, C], F32)
                    n_mm = 12 + 7 + 3 + 1
                    mi = 0
                    # L0: 4 chunks x 3 slots
                    for c in range(L0['NCH']):
                        for s in range(3):
                            w = wp.tile([128, NBIN], F32, tag="w")
                            colw = WCOL0_OFF + roi * 12 + c * 3 + s
                            nc.vector.tensor_scalar_mul(
                                w[:],
                                cst_t[:, PAT0_OFF + c * NBIN:PAT0_OFF + (c + 1) * NBIN],
                                cst_t[:, colw:colw + 1])
                            nc.tensor.matmul(
                                acc[:], w[:],
                                gts[0][:, r2 * L0['NCH'] + c, s * C:(s + 1) * C],
                                start=(mi == 0), stop=(mi == n_mm - 1))
                            mi += 1
                    # L1: 7 chunks
                    for c in range(L1['NCH']):
                        w = wp.tile([128, NBIN], F32, tag="w")
                        colw = WCOL1_OFF + roi * 7 + c
                        nc.vector.tensor_scalar_mul(
                            w[:],
                            cst_t[:, PAT1_OFF + c * NBIN:PAT1_OFF + (c + 1) * NBIN],
                            cst_t[:, colw:colw + 1])
                        nc.tensor.matmul(
                            acc[:], w[:], gts[1][:, r2 * L1['NCH'] + c, :],
                            start=(mi == 0), stop=(mi == n_mm - 1))
                        mi += 1
                    # L2: 3 chunks, host-baked lhsT
                    for c in range(L2['NCH']):
                        nc.tensor.matmul(
                            acc[:], lt2_t[:, c, :], gts[2][:, r2 * L2['NCH'] + c, :],
                            start=(mi == 0), stop=(mi == n_mm - 1))
                        mi += 1
                    # L3: 1 chunk
                    nc.tensor.matmul(
                        acc[:], lt3_t[:], gts[3][:, r2, :],
                        start=(mi == 0), stop=(mi == n_mm - 1))
                    mi += 1

                    ev = evp.tile([NBIN, C], F32, tag="ev")
                    nc.scalar.copy(ev[:], acc[:])
                    pt = ptp.tile([128, 2, NBIN], F32, tag="pt")
                    for h in range(2):
                        nc.tensor.transpose(
                            pt[:, h, :], ev[:, h * 128:(h + 1) * 128],
                            cst_t[:NBIN, ID_OFF:ID_OFF + NBIN])
                    ot = otp.tile([128, 2, NBIN], F32, tag="ot")
                    nc.vector.tensor_copy(ot[:], pt[:])
                    # out[roi] is [256, 49]; view as [h, p, m] -> dst [p, h, m]
                    dst = out[roi].rearrange("(h p) m -> p h m", h=2)
                    nc.sync.dma_start(dst, ot[:])
    nc.finalize()
    return nc


def kernel(x0, x1, x2, x3, boxes):
    from concourse.bass_utils import run_bass_kernel_spmd
    in_maps = _host_prepare(x0, x1, x2, x3, boxes)
    if 'nc' not in _MODULE_CACHE:
        _MODULE_CACHE['nc'] = _build_module()
    nc = _MODULE_CACHE['nc']
    res = run_bass_kernel_spmd(nc, in_maps, list(range(8)))
    _MODULE_CACHE['last_res'] = res
    outs = [res.results[k]["out"] for k in range(8)]
    full = np.concatenate(outs, axis=0)           # [1024, 256, 49]
    return full.reshape(1024, C, POOLED, POOLED).astype(np.float32)



# revision 13
# speedup vs baseline: 1.2531x; 1.2531x over previous
"""Multi-level ROI Align (FPN pooler, 4 levels summed) on 8 Trainium2 cores.

Strategy: shard ROIs across cores (core k: batch k//4, 128 ROIs). All gather
indices and bilinear weights are computed on host from `boxes`; the device
kernel does HBM pixel gathers (SWDGE dma_gather, prepare_only + trigger so
desc-gen overlaps the transfers) + weighted reduction into 7x7 bins via
PSUM-accumulating bf16 matmuls. Output [49, C] is DMA'd straight from PSUM;
the host does the final [49,C] -> [C,7,7] transpose.

Per ROI, per level:  out[bin, c] = sum_j W[j, bin] * G[j, c]
where G rows are gathered pixel vectors (C=256, bf16) and W is sparse, built
on device as fixed_pattern * per-partition scalar (L0/L1; one fused
broadcast-multiply per ROI) or host-baked dense (region levels L2/L3).

L0 uses 3-px elements addressed at even-pixel granularity (idx = flat//2) to
fit the int16 index range (200*200 = 40000 > 32767); 3 weight slots.
L1 uses 2-px pair elements at 1-px granularity (10000 fits int16); 2 slots.
"""
import sys
import numpy as np
import ml_dtypes

sys.path.insert(0, '/opt/trn_rl_repo')

BF16 = ml_dtypes.bfloat16

POOLED = 7
SAMP = 2
NBIN = 49
C = 256
IMG = 800.0

# per level: H, W, scale, mode
#   mode 'tri' : 3-px elems, idx=flat//2, 4 chunks x 3 slots
#   mode 'pair': 2-px elems, idx=flat, 4 chunks x 2 slots
#   mode 'reg' : 1-px elems, bounding-region pixels, host-baked lhsT
L0 = dict(H=200, W=200, scale=0.25, mode='tri', NJ=512, REAL=392, NCH=4, NSLOT=3)
L1 = dict(H=100, W=100, scale=0.125, mode='pair', NJ=512, REAL=392, NCH=4, NSLOT=2)
L2 = dict(H=50, W=50, scale=0.0625, mode='reg', NJ=384, REAL=324, NCH=3, WREG=18)
L3 = dict(H=25, W=25, scale=0.03125, mode='reg', NJ=128, REAL=100, NCH=1, WREG=10)
LEVELS = [L0, L1, L2, L3]

NROI_CORE = 128     # ROIs per core
NGRP = 64           # groups of 2 ROIs
GRP = 2

# padded flat pixel counts of the feature buffers
F0_ROWS = 40004     # covers 3-px elem overrun (idx 19999 -> px 39998..40001)
F1_ROWS = 10002     # covers 2-px pair overrun (idx 9999 -> px 9999..10000)
F2_ROWS = 3400      # covers region overrun (y,x up to 66)
F3_ROWS = 900       # covers region overrun (y,x up to 33)

# const bf16 column layout (per partition)
PAT_OFF = 0                        # [4, 49] shared one-hot (same j order L0/L1)
WCOL0_OFF = PAT_OFF + 4 * NBIN     # [128 roi * 12]  (c4 x s3)
WCOL1_OFF = WCOL0_OFF + NROI_CORE * 12   # [128 roi * 8]  (c4 x s2)
CST_COLS = WCOL1_OFF + NROI_CORE * 8

# idx int16 column layout (per partition), per 2-ROI group
IC0, IC1, IC2, IC3 = 64, 64, 48, 16     # cols per group per level
IDX0_OFF = 0
IDX1_OFF = IDX0_OFF + NGRP * IC0
IDX2_OFF = IDX1_OFF + NGRP * IC1
IDX3_OFF = IDX2_OFF + NGRP * IC2
IDX_COLS = IDX3_OFF + NGRP * IC3

PREP = False         # prepare_only + trigger_dma gathers (async) vs synchronous

_MODULE_CACHE = {}


def _sample_meta(boxes_b, H, W, scale):
    """Per-ROI sample geometry in fp32, matching reference op order.
    boxes_b: [N, 4] fp32. Returns dict of [N,7,2] arrays."""
    f = np.float32
    b = boxes_b.astype(np.float32)
    x1 = b[:, 0] * f(scale)
    y1 = b[:, 1] * f(scale)
    x2 = b[:, 2] * f(scale)
    y2 = b[:, 3] * f(scale)
    rw = np.maximum(x2 - x1, f(1.0))
    rh = np.maximum(y2 - y1, f(1.0))
    bw = rw / f(POOLED)
    bh = rh / f(POOLED)
    g = (np.arange(POOLED, dtype=np.float32)[:, None]
         + (np.arange(SAMP, dtype=np.float32)[None, :] + f(0.5)) / f(SAMP))
    y = y1[:, None, None] + g[None] * bh[:, None, None]   # [N,7,2]
    x = x1[:, None, None] + g[None] * bw[:, None, None]
    masky = ((y >= f(-1.0)) & (y <= f(H))).astype(np.float32)
    maskx = ((x >= f(-1.0)) & (x <= f(W))).astype(np.float32)
    yc = np.clip(y, f(0.0), f(H - 1))
    xc = np.clip(x, f(0.0), f(W - 1))
    yl = np.floor(yc).astype(np.int64)
    xl = np.floor(xc).astype(np.int64)
    yh = np.minimum(yl + 1, H - 1)
    xh = np.minimum(xl + 1, W - 1)
    ly = (yc - yl.astype(np.float32)).astype(np.float32)
    lx = (xc - xl.astype(np.float32)).astype(np.float32)
    hy = (f(1.0) - ly).astype(np.float32)
    hx = (f(1.0) - lx).astype(np.float32)
    return dict(yl=yl, yh=yh, xl=xl, xh=xh, ly=ly, lx=lx, hy=hy, hx=hx,
                masky=masky, maskx=maskx, x=x, y=y)


def _build_tri(meta, lv):
    """L0: j = (row_sel, py, sy, px, sx) -> 392 3-px elems, 3 slot weights.
    Returns idx [N, NJ] int64, w [N, NJ, 3] fp32."""
    N = meta['yl'].shape[0]
    W = lv['W']
    NJ, REAL = lv['NJ'], lv['REAL']
    rows = np.stack([meta['yl'], meta['yh']], axis=1)          # [N,2,7,2] (rs)
    wys = np.stack([meta['hy'], meta['ly']], axis=1)           # [N,2,7,2]
    m = (meta['masky'][:, :, :, None, None] * meta['maskx'][:, None, None, :, :])  # [N,7,2,7,2]
    row = np.broadcast_to(rows[:, :, :, :, None, None], (N, 2, 7, 2, 7, 2))
    wy = np.broadcast_to(wys[:, :, :, :, None, None], (N, 2, 7, 2, 7, 2)).astype(np.float32)
    xl = np.broadcast_to(meta['xl'][:, None, None, None, :, :], (N, 2, 7, 2, 7, 2))
    hx = np.broadcast_to(meta['hx'][:, None, None, None, :, :], (N, 2, 7, 2, 7, 2)).astype(np.float32)
    lx = np.broadcast_to(meta['lx'][:, None, None, None, :, :], (N, 2, 7, 2, 7, 2)).astype(np.float32)
    mm = np.broadcast_to(m[:, None], (N, 2, 7, 2, 7, 2)).astype(np.float32)
    flat = row * W + xl
    idx = (flat >> 1).reshape(N, REAL)
    r = (flat & 1).astype(np.float32).reshape(N, REAL)
    wl = (wy * hx * mm * np.float32(0.25)).reshape(N, REAL)
    wh = (wy * lx * mm * np.float32(0.25)).reshape(N, REAL)
    w = np.zeros((N, NJ, 3), np.float32)
    w[:, :REAL, 0] = wl * (1 - r)
    w[:, :REAL, 1] = wl * r + wh * (1 - r)
    w[:, :REAL, 2] = wh * r
    idx_full = np.zeros((N, NJ), np.int64)
    idx_full[:, :REAL] = idx
    return idx_full, w


def _build_pair(meta, lv):
    """L1: j = (row_sel, py, sy, px, sx) -> 392 2-px pair elems, 2 slot weights.
    Pair at (row, xl) covers xl and xl+1; when xl == W-1 the second pixel is
    invalid and its weight folds into slot 0 (xh == xl in the reference).
    Returns idx [N, NJ] int64, w [N, NJ, 2] fp32."""
    N = meta['yl'].shape[0]
    W = lv['W']
    NJ, REAL = lv['NJ'], lv['REAL']
    rows = np.stack([meta['yl'], meta['yh']], axis=1)          # [N,2,7,2]
    wys = np.stack([meta['hy'], meta['ly']], axis=1)
    m = (meta['masky'][:, :, :, None, None] * meta['maskx'][:, None, None, :, :])
    row = np.broadcast_to(rows[:, :, :, :, None, None], (N, 2, 7, 2, 7, 2))
    wy = np.broadcast_to(wys[:, :, :, :, None, None], (N, 2, 7, 2, 7, 2)).astype(np.float32)
    xl = np.broadcast_to(meta['xl'][:, None, None, None, :, :], (N, 2, 7, 2, 7, 2))
    hx = np.broadcast_to(meta['hx'][:, None, None, None, :, :], (N, 2, 7, 2, 7, 2)).astype(np.float32)
    lx = np.broadcast_to(meta['lx'][:, None, None, None, :, :], (N, 2, 7, 2, 7, 2)).astype(np.float32)
    mm = np.broadcast_to(m[:, None], (N, 2, 7, 2, 7, 2)).astype(np.float32)
    idx = (row * W + xl).reshape(N, REAL)
    edge = (xl == W - 1)
    w0 = (wy * hx * mm * np.float32(0.25))
    w1 = (wy * lx * mm * np.float32(0.25))
    w0 = np.where(edge, w0 + w1, w0).reshape(N, REAL)
    w1 = np.where(edge, np.float32(0.0), w1).reshape(N, REAL)
    w = np.zeros((N, NJ, 2), np.float32)
    w[:, :REAL, 0] = w0
    w[:, :REAL, 1] = w1
    idx_full = np.zeros((N, NJ), np.int64)
    idx_full[:, :REAL] = idx
    return idx_full, w


def _build_reg(meta, lv):
    """L2/L3: bounding-region pixels + separable host-baked weights.
    Returns idx [N, NJ] int64, lhsT [N, NJ, 49] fp32."""
    N = meta['yl'].shape[0]
    H, W, WREG = lv['H'], lv['W'], lv['WREG']
    NJ, REAL = lv['NJ'], lv['REAL']
    f = np.float32
    y_base = np.floor(np.clip(meta['y'].reshape(N, -1).min(1), 0.0, H - 1)).astype(np.int64)
    x_base = np.floor(np.clip(meta['x'].reshape(N, -1).min(1), 0.0, W - 1)).astype(np.int64)
    WY = np.zeros((N, WREG, POOLED), np.float32)
    WX = np.zeros((N, WREG, POOLED), np.float32)
    ridx = np.arange(N)[:, None, None]
    pidx = np.broadcast_to(np.arange(POOLED)[None, :, None], (N, POOLED, SAMP))
    np.add.at(WY, (ridx, meta['yl'] - y_base[:, None, None], pidx),
              (f(0.5) * meta['hy'] * meta['masky']).astype(np.float32))
    np.add.at(WY, (ridx, meta['yh'] - y_base[:, None, None], pidx),
              (f(0.5) * meta['ly'] * meta['masky']).astype(np.float32))
    np.add.at(WX, (ridx, meta['xl'] - x_base[:, None, None], pidx),
              (f(0.5) * meta['hx'] * meta['maskx']).astype(np.float32))
    np.add.at(WX, (ridx, meta['xh'] - x_base[:, None, None], pidx),
              (f(0.5) * meta['lx'] * meta['maskx']).astype(np.float32))
    lhsT = np.einsum('nap,nbq->nabpq', WY, WX).reshape(N, REAL, NBIN)
    dy = np.arange(WREG)
    idx = ((y_base[:, None, None] + dy[None, :, None]) * W
           + x_base[:, None, None] + dy[None, None, :]).reshape(N, REAL)
    idx_full = np.zeros((N, NJ), np.int64)
    lhsT_full = np.zeros((N, NJ, NBIN), np.float32)
    idx_full[:, :REAL] = idx
    lhsT_full[:, :REAL] = lhsT
    return idx_full, lhsT_full


def _pack_idx(jlists):
    """Pack concatenated per-group idx list [NJ_total] -> [128, NJ_total//16]
    int16 wrapped in 16 partitions, replicated 8x."""
    jl = np.asarray(jlists)
    n = jl.shape[-1]
    arr = jl.reshape(*jl.shape[:-1], n // 16, 16)   # [..., col, p]
    arr = np.swapaxes(arr, -1, -2)                  # [..., p(16), col]
    arr = np.broadcast_to(arr[..., None, :, :],
                          (*jl.shape[:-1], 8, 16, n // 16))
    return arr.reshape(*jl.shape[:-1], 128, n // 16).astype(np.int16)


def _bin_pattern(REAL):
    """Fixed j->bin one-hot pattern [128, 4, 49] for (rs,py,sy,px,sx) j order."""
    NJ = 4 * 128
    j = np.arange(NJ)
    px = (j // 2) % 7
    py = (j // (2 * 7 * 2)) % 7
    bins = py * 7 + px
    pat = np.zeros((NJ, NBIN), np.float32)
    valid = j < REAL
    pat[np.arange(NJ)[valid], bins[valid]] = 1.0
    return pat.reshape(4, 128, NBIN).transpose(1, 0, 2)   # [128, 4, 49]


def _host_prepare(x0, x1, x2, x3, boxes):
    """Build all per-core input tensors. Returns list of 8 dicts."""
    B = boxes.shape[0]
    feats = []
    for arr, lv, rows in ((x0, L0, F0_ROWS), (x1, L1, F1_ROWS),
                          (x2, L2, F2_ROWS), (x3, L3, F3_ROWS)):
        f = np.zeros((B, rows, C), BF16)
        hw = lv['H'] * lv['W']
        f[:, :hw] = np.ascontiguousarray(
            np.transpose(np.asarray(arr, np.float32), (0, 2, 3, 1))).reshape(B, hw, C).astype(BF16)
        feats.append(f)

    per_batch = []
    for b in range(B):
        bb = np.asarray(boxes[b], np.float32)
        m0 = _sample_meta(bb, L0['H'], L0['W'], L0['scale'])
        m1 = _sample_meta(bb, L1['H'], L1['W'], L1['scale'])
        m2 = _sample_meta(bb, L2['H'], L2['W'], L2['scale'])
        m3 = _sample_meta(bb, L3['H'], L3['W'], L3['scale'])
        idx0, w0 = _build_tri(m0, L0)
        idx1, w1 = _build_pair(m1, L1)
        idx2, lt2 = _build_reg(m2, L2)
        idx3, lt3 = _build_reg(m3, L3)
        per_batch.append((idx0, w0, idx1, w1, idx2, lt2, idx3, lt3))

    pat = _bin_pattern(392)      # [128, 4, 49] shared by L0 (x3 slots) and L1 (x2)

    in_maps = []
    for k in range(8):
        b = k // 4
        s = (k % 4) * NROI_CORE
        idx0, w0, idx1, w1, idx2, lt2, idx3, lt3 = per_batch[b]
        sl = slice(s, s + NROI_CORE)

        cst = np.zeros((128, CST_COLS), BF16)
        cst[:, PAT_OFF:PAT_OFF + 4 * NBIN] = pat.reshape(128, -1).astype(BF16)
        # wcol0 [128, roi*12]: col roi*12 + c*3 + slot = w0[roi, c*128+p, slot]
        wc0 = w0[sl].reshape(NROI_CORE, 4, 128, 3)           # [roi,c,p,s]
        cst[:, WCOL0_OFF:WCOL0_OFF + NROI_CORE * 12] = (
            wc0.transpose(2, 0, 1, 3).reshape(128, -1).astype(BF16))
        wc1 = w1[sl].reshape(NROI_CORE, 4, 128, 2)           # [roi,c,p,s]
        cst[:, WCOL1_OFF:WCOL1_OFF + NROI_CORE * 8] = (
            wc1.transpose(2, 0, 1, 3).reshape(128, -1).astype(BF16))

        # per-group idx lists with -1 tail for roi_b (groups >= 2)
        def glists(idx_full, lv):
            out = []
            for grp in range(NGRP):
                a = idx_full[s + grp * 2].copy()
                bb_ = idx_full[s + grp * 2 + 1].copy()
                if grp >= 2:
                    bb_[lv['REAL']:] = -1
                out.append(np.concatenate([a, bb_]))
            return np.stack(out)                              # [NGRP, 2*NJ]

        idxs = np.zeros((128, IDX_COLS), np.int16)
        idxs[:, IDX0_OFF:IDX0_OFF + NGRP * IC0] = _pack_idx(
            glists(idx0, L0)).transpose(1, 0, 2).reshape(128, -1)
        idxs[:, IDX1_OFF:IDX1_OFF + NGRP * IC1] = _pack_idx(
            glists(idx1, L1)).transpose(1, 0, 2).reshape(128, -1)
        idxs[:, IDX2_OFF:IDX2_OFF + NGRP * IC2] = _pack_idx(
            glists(idx2, L2)).transpose(1, 0, 2).reshape(128, -1)
        idxs[:, IDX3_OFF:IDX3_OFF + NGRP * IC3] = _pack_idx(
            glists(idx3, L3)).transpose(1, 0, 2).reshape(128, -1)

        # lhsT k-major: lt2 [roi, NJ(=3*128), 49] -> [roi, 128, 3, 49]
        lt2k = np.ascontiguousarray(
            lt2[sl].reshape(NROI_CORE, L2['NCH'], 128, NBIN).transpose(0, 2, 1, 3)).astype(BF16)
        lt3k = np.ascontiguousarray(lt3[sl].reshape(NROI_CORE, 128, NBIN)).astype(BF16)

        in_maps.append({
            "f0": feats[0][b], "f1": feats[1][b],
            "f2": feats[2][b], "f3": feats[3][b],
            "cst": cst, "idxs": idxs, "lt2": lt2k, "lt3": lt3k,
        })
    return in_maps


def _patch_prep_sems(nc):
    """Point each gen_mode==1 SWDGE prep's descriptor sem (on_update[0]) at the
    DMASW lane sem tile assigned it (round-robin over 8 lanes, program order).
    Tile attaches consumer waits on the lane sems but does not rewrite the
    prep's own sem= — without this patch nothing increments the lane sems."""
    import re
    import bass_rust
    fn = nc.m.functions[0]
    lane_ids = {}
    insts = []
    for bb in fn.blocks:
        for ins in bb.instructions:
            insts.append(ins)
            si = ins.sync_info
            if not si:
                continue
            for ent in list(si.on_wait or []) + list(si.on_update or []):
                m = re.match(r"DMASW(\d+)_", ent.ant_name or "")
                if m:
                    lane_ids[int(m.group(1))] = ent.id
    k = 0
    for ins in insts:
        if type(ins).__name__ in ('InstDMAGatherAnt', 'InstDMAScatterAddAnt') \
                and getattr(ins, 'gen_mode', 0) == 1:
            lane = k % 8
            assert lane in lane_ids, f"lane {lane} sem missing ({sorted(lane_ids)})"
            si = ins.sync_info
            u0 = si.on_update[0]
            nu = bass_rust.SyncUpdate(id=lane_ids[lane], update_value=16,
                                      sync_type=u0.sync_type,
                                      update_mode=u0.update_mode)
            lst = si.on_update
            lst[0] = nu
            si.on_update = lst
            ins.sync_info = si
            k += 1
    return k


def _build_module():
    from concourse import bacc, tile
    from concourse.bass import mybir
    import concourse.bass as bass_mod

    F32 = mybir.dt.float32
    BF = mybir.dt.bfloat16
    I16 = mybir.dt.int16
    AP = bass_mod.AP

    nc = bacc.Bacc(None, target_bir_lowering=False)
    f0 = nc.dram_tensor("f0", [F0_ROWS, C], BF, kind="ExternalInput")
    f1 = nc.dram_tensor("f1", [F1_ROWS, C], BF, kind="ExternalInput")
    f2 = nc.dram_tensor("f2", [F2_ROWS, C], BF, kind="ExternalInput")
    f3 = nc.dram_tensor("f3", [F3_ROWS, C], BF, kind="ExternalInput")
    cst = nc.dram_tensor("cst", [128, CST_COLS], BF, kind="ExternalInput")
    idxs = nc.dram_tensor("idxs", [128, IDX_COLS], I16, kind="ExternalInput")
    lt2 = nc.dram_tensor("lt2", [NROI_CORE, 128, L2['NCH'], NBIN], BF, kind="ExternalInput")
    lt3 = nc.dram_tensor("lt3", [NROI_CORE, 128, NBIN], BF, kind="ExternalInput")
    out = nc.dram_tensor("out", [NROI_CORE, NBIN, C], F32, kind="ExternalOutput")

    # overlapping 3-px elem view of f0 (stride 2px) / 2-px pair view of f1 (stride 1px)
    f0_view = AP(f0, 0, [[2 * C, F0_ROWS // 2 - 1], [1, 3 * C]])
    f1_view = AP(f1, 0, [[C, F1_ROWS - 1], [1, 2 * C]])
    gather_srcs = [f0_view, f1_view, f2[:], f3[:]]
    ELEM = [3 * C, 2 * C, C, C]
    STEP = [2 * C, C, C, C]
    ICOLS = [IC0, IC1, IC2, IC3]
    IOFF = [IDX0_OFF, IDX1_OFF, IDX2_OFF, IDX3_OFF]
    NIDX = [GRP * L0['NJ'], GRP * L1['NJ'], GRP * L2['NJ'], GRP * L3['NJ']]
    NCHG = [GRP * L0['NCH'], GRP * L1['NCH'], GRP * L2['NCH'], GRP * L3['NCH']]

    def bc(ap_base, dims):
        """AP with explicit [stride, num] dims on top of a sliced tile AP."""
        return AP(ap_base.tensor, ap_base.offset, dims)

    with tile.TileContext(nc) as tc:
        with (
            tc.tile_pool(name="const", bufs=1) as constp,
            tc.tile_pool(name="g0p", bufs=2) as g0p,
            tc.tile_pool(name="g1p", bufs=2) as g1p,
            tc.tile_pool(name="g2p", bufs=2) as g2p,
            tc.tile_pool(name="g3p", bufs=2) as g3p,
            tc.tile_pool(name="ltp", bufs=3) as ltp,
            tc.tile_pool(name="wp", bufs=4) as wp,
            tc.tile_pool(name="accp", bufs=4, space="PSUM") as accp,
            tc.tile_pool(name="evp", bufs=3) as evp,
        ):
            cst_t = constp.tile([128, CST_COLS], BF)
            nc.sync.dma_start(cst_t[:], cst[:])
            idx_t = constp.tile([128, IDX_COLS], I16)
            nc.sync.dma_start(idx_t[:], idxs[:])

            dma_sem = nc.alloc_semaphore("gsem")
            # helper NEFFs sharing the device may leave the sem nonzero;
            # a dirty initial value lets early waits pass before data lands.
            nc.gpsimd.dma_reset(range(dma_sem.num, dma_sem.num + 1))
            nc.gpsimd.sem_clear(dma_sem)

            gpools = [g0p, g1p, g2p, g3p]
            for grp in range(NGRP):
                gts = []
                for l, lv in enumerate(LEVELS):
                    gt = gpools[l].tile([128, NCHG[l], ELEM[l]], BF, tag=f"g{l}")
                    io = IOFF[l] + grp * ICOLS[l]
                    if PREP:
                        nc.gpsimd.dma_gather(
                            gt[:], gather_srcs[l], idx_t[:, io:io + ICOLS[l]],
                            NIDX[l], NIDX[l], ELEM[l], elem_step=STEP[l],
                            prepare_only=True, sem=dma_sem)
                        nc.gpsimd.trigger_dma(count=None)
                    else:
                        nc.gpsimd.dma_gather(
                            gt[:], gather_srcs[l], idx_t[:, io:io + ICOLS[l]],
                            NIDX[l], NIDX[l], ELEM[l], elem_step=STEP[l])
                    gts.append(gt)

                for r2 in range(GRP):
                    roi = grp * GRP + r2
                    lt2_t = ltp.tile([128, L2['NCH'], NBIN], BF, tag="lt2")
                    nc.sync.dma_start(lt2_t[:], lt2[roi])
                    lt3_t = ltp.tile([128, NBIN], BF, tag="lt3")
                    nc.sync.dma_start(lt3_t[:], lt3[roi])

                    # fused weight build: w = one_hot_pattern * per-(chunk,slot) scalar
                    pat_ap = cst_t[:, PAT_OFF:PAT_OFF + 4 * NBIN]
                    ps = pat_ap.ap[0][0]
                    w0 = wp.tile([128, 12, NBIN], BF, tag="w0")
                    wc0_ap = cst_t[:, WCOL0_OFF + roi * 12:WCOL0_OFF + (roi + 1) * 12]
                    nc.vector.tensor_mul(
                        bc(w0[:], [[w0[:].ap[0][0], 128], [3 * NBIN, 4], [NBIN, 3], [1, NBIN]]),
                        bc(pat_ap, [[ps, 128], [NBIN, 4], [0, 3], [1, NBIN]]),
                        bc(wc0_ap, [[ps, 128], [3, 4], [1, 3], [0, NBIN]]))
                    w1 = wp.tile([128, 8, NBIN], BF, tag="w1")
                    wc1_ap = cst_t[:, WCOL1_OFF + roi * 8:WCOL1_OFF + (roi + 1) * 8]
                    nc.vector.tensor_mul(
                        bc(w1[:], [[w1[:].ap[0][0], 128], [2 * NBIN, 4], [NBIN, 2], [1, NBIN]]),
                        bc(pat_ap, [[ps, 128], [NBIN, 4], [0, 2], [1, NBIN]]),
                        bc(wc1_ap, [[ps, 128], [2, 4], [1, 2], [0, NBIN]]))

                    acc = accp.tile([NBIN, C], F32)
                    n_mm = 12 + 8 + 3 + 1
                    mi = 0
                    for c in range(L0['NCH']):
                        for sslot in range(3):
                            nc.tensor.matmul(
                                acc[:], w0[:, c * 3 + sslot, :],
                                gts[0][:, r2 * L0['NCH'] + c, sslot * C:(sslot + 1) * C],
                                start=(mi == 0), stop=(mi == n_mm - 1))
                            mi += 1
                    for c in range(L1['NCH']):
                        for sslot in range(2):
                            nc.tensor.matmul(
                                acc[:], w1[:, c * 2 + sslot, :],
                                gts[1][:, r2 * L1['NCH'] + c, sslot * C:(sslot + 1) * C],
                                start=(mi == 0), stop=(mi == n_mm - 1))
                            mi += 1
                    for c in range(L2['NCH']):
                        nc.tensor.matmul(
                            acc[:], lt2_t[:, c, :], gts[2][:, r2 * L2['NCH'] + c, :],
                            start=(mi == 0), stop=(mi == n_mm - 1))
                        mi += 1
                    nc.tensor.matmul(
                        acc[:], lt3_t[:], gts[3][:, r2, :],
                        start=(mi == 0), stop=(mi == n_mm - 1))
                    mi += 1

                    ev = evp.tile([NBIN, C], F32, tag="ev")
                    nc.scalar.copy(ev[:], acc[:])
                    nc.sync.dma_start(out[roi], ev[:])
    nc.finalize()
    _patch_prep_sems(nc)
    return nc


def kernel(x0, x1, x2, x3, boxes):
    from concourse.bass_utils import run_bass_kernel_spmd
    in_maps = _host_prepare(x0, x1, x2, x3, boxes)
    if 'nc' not in _MODULE_CACHE:
        _MODULE_CACHE['nc'] = _build_module()
    nc = _MODULE_CACHE['nc']
    # Execute twice: semaphores may hold residue from other NEFFs that ran on
    # the device (e.g. jax input generation); the first execution's epilogue
    # range-clears the tile semaphores, so the second starts clean.
    run_bass_kernel_spmd(nc, in_maps, list(range(8)))
    res = run_bass_kernel_spmd(nc, in_maps, list(range(8)))
    _MODULE_CACHE['last_res'] = res
    outs = [res.results[k]["out"] for k in range(8)]          # each [128, 49, 256]
    full = np.concatenate(outs, axis=0)                       # [1024, 49, 256]
    full = np.ascontiguousarray(full.transpose(0, 2, 1))      # [1024, 256, 49]
    return full.reshape(1024, C, POOLED, POOLED).astype(np.float32)
